# revision 1
# baseline (speedup 1.0000x reference)
"""Trainium2 Bass kernel for nn_BaseSparseConn (gnn_message_passing).

Computes out = x @ conn + bias where conn is given in COO form
(rows = dst, cols = src of the transposed matrix):
    out.T[r, :] = sum_{e: rows[e]==r} values[e] * x[:, cols[e]]  + bias[r]

Strategy (8 NeuronCores, SPMD — one NEFF, per-core data):
  - Row-partition the output: core c owns output rows [c*12500, (c+1)*12500).
  - Per core, rows are processed in 98 blocks of 128 rows.  A block's edges
    (avg ~2048) are fetched with dma_gather (SWDGE) from a zero-padded fp16
    copy of x^T laid out as (IN_F, 128) so each gather element is 256 B.
    dma_gather requires int16 indices, so each block's edges are bucketed
    into 4 column ranges of 25000 and padded to a fixed chunk count.
  - Scatter-add into the 128 output rows of a block is a one-hot matmul:
    one batched DVE tensor_tensor builds M_eq[p, kk, m] = (rows[p,kk] == m)
    per block, values are multiplied into the gathered data in place (one
    DVE op per range covering the whole group), and the PE accumulates
    psum[128 rows, 64 batch] += M_eq[:,kk,:].T @ gathered across chunks.
    Gathers run on SWDGE queues 0-3 so four Q7 core pairs generate
    descriptors concurrently.
  - Bias is a final rank-1 matmul into PSUM; the Scalar engine copies
    PSUM->SBUF and the result is DMA'd out.
"""

import numpy as np

# Problem constants (hardcoded per the harness contract)
B = 64
IN_F = 100000
OUT_F = 100000
N_CORES = 8

# Sharding / layout constants
ROWS_PER_CORE = OUT_F // N_CORES  # 12500
BLK = 128
GROUP = 7                         # blocks per gather group (98 = 14*7)
N_RANGES = 4
RANGE_W = 25000                   # int16 gather index bound (< 32768)
XPAD = 128                        # padded batch so gather elem = 256 B fp16


def _cdiv(a, b):
    return -(-a // b)


class Cfg:
    """Geometry shared between host-side data prep and the device program."""

    def __init__(self, in_f, out_f, batch, n_cores, rows_per_core, group,
                 n_ranges, range_w, cpr, xpad=128, blk=128):
        assert range_w <= 32768
        assert rows_per_core % blk == 0 or True
        self.in_f = in_f
        self.out_f = out_f
        self.batch = batch
        self.n_cores = n_cores
        self.rows_per_core = rows_per_core
        self.blk = blk
        self.group = group
        self.n_ranges = n_ranges
        self.range_w = range_w
        assert n_ranges * range_w >= in_f
        self.cpr = cpr                        # chunks per (block, range)
        self.xpad = xpad
        self.n_blocks = _cdiv(rows_per_core, blk)       # blocks per core
        assert self.n_blocks % group == 0, (self.n_blocks, group)
        self.n_groups = self.n_blocks // group
        self.cpt = n_ranges * cpr             # chunks per block
        self.slots_pg = n_ranges * group * cpr  # gather slots per group
        self.idx_w = self.slots_pg * 8        # idx free-dim per group (int16)
        self.rv_w = group * 2 * self.cpt      # rows+vals free-dim per group
        self.out_rows = self.n_blocks * blk   # padded output rows per core


def prep_host_data(cfg, x, values, bias, rows, cols):
    """Shard + lay out inputs for the device program.

    Returns (shared_inputs, per_core_inputs).
    """
    rows = np.asarray(rows).astype(np.int64)
    cols = np.asarray(cols).astype(np.int64)
    values = np.asarray(values, dtype=np.float32)
    bias = np.asarray(bias, dtype=np.float32)
    x = np.asarray(x, dtype=np.float32)

    # zero-padded fp16 x^T: row i = x[:, i] padded to xpad columns
    xp = np.zeros((cfg.in_f, cfg.xpad), dtype=np.float16)
    xp[:, :cfg.batch] = x.T.astype(np.float16)

    iota = np.tile(np.arange(128, dtype=np.float16), (128, 1))

    per_core = []
    for c in range(cfg.n_cores):
        e0, e1 = np.searchsorted(rows, [c * cfg.rows_per_core,
                                        (c + 1) * cfg.rows_per_core])
        r_loc = (rows[e0:e1] - c * cfg.rows_per_core).astype(np.int64)
        col = cols[e0:e1]
        val = values[e0:e1]

        blk_id = r_loc // cfg.blk
        rng_id = col // cfg.range_w
        key = blk_id * cfg.n_ranges + rng_id
        order = np.argsort(key, kind="stable")
        key_s = key[order]
        col_s = col[order]
        val_s = val[order]
        row_s = (r_loc - blk_id * cfg.blk)[order]       # 0..127 within block

        counts = np.bincount(key_s, minlength=cfg.n_blocks * cfg.n_ranges)
        limit = cfg.cpr * 128
        assert counts.max() <= limit, (counts.max(), limit)
        starts = np.concatenate([[0], np.cumsum(counts)[:-1]])
        # position of each edge within its (block, range) bucket
        q = np.arange(len(key_s)) - starts[key_s]

        b_s = key_s // cfg.n_ranges
        r_s = key_s % cfg.n_ranges
        g_s = b_s // cfg.group
        j_s = b_s % cfg.group

        # ---- gather index array, 8x replicated across the 128 partitions.
        # One gather per (group, range): batch of group*cpr*128 indices,
        # element i lives at [i % 16, base + i // 16].
        npart_w = cfg.group * cfg.cpr * 128 // 16        # per-range free width
        # Padding slots repeat the bucket's last real index (HBM row-hit
        # instead of a cold read of row 0); value is 0 so they contribute 0.
        pad_idx = np.zeros((cfg.n_groups, cfg.n_ranges, cfg.group, cfg.cpr * 128),
                           dtype=np.int16)
        lastidx = np.zeros(cfg.n_groups * cfg.n_ranges * cfg.group,
                           dtype=np.int16)
        flatkey = (g_s * cfg.n_ranges + r_s) * cfg.group + j_s
        lastidx[flatkey] = (col_s - r_s * cfg.range_w).astype(np.int16)
        pad_idx[:] = lastidx.reshape(cfg.n_groups, cfg.n_ranges,
                                     cfg.group)[..., None]
        pad_idx = pad_idx.reshape(cfg.n_groups, cfg.n_ranges,
                                  cfg.group * cfg.cpr * 128)
        # scatter into the 16-partition wrap layout
        idx16 = np.empty((cfg.n_groups, 16, cfg.n_ranges * npart_w),
                         dtype=np.int16)
        for rr in range(cfg.n_ranges):
            blk16 = pad_idx[:, rr].reshape(cfg.n_groups, npart_w, 16)
            idx16[:, :, rr * npart_w:(rr + 1) * npart_w] = \
                blk16.transpose(0, 2, 1)
        i_in_gather = j_s * (cfg.cpr * 128) + q
        idx16[g_s, i_in_gather % 16,
              r_s * npart_w + i_in_gather // 16] = (col_s - r_s * cfg.range_w
                                                    ).astype(np.int16)
        idx_full = np.tile(idx16, (1, 8, 1))             # replicate to 128 parts

        # ---- rows/vals array: (n_groups, 128, rv_w) fp16
        rv = np.zeros((cfg.n_groups, 128, cfg.rv_w), dtype=np.float16)
        kk = r_s * cfg.cpr + q // 128                    # chunk id in block
        p = q % 128
        rv[g_s, p, j_s * 2 * cfg.cpt + kk] = row_s.astype(np.float16)
        rv[g_s, p, j_s * 2 * cfg.cpt + cfg.cpt + kk] = val_s.astype(np.float16)

        # ---- bias array: (n_groups, 1, group*blk) fp16 (rank-1 matmul row)
        bias_arr = np.zeros((cfg.n_groups, 1, cfg.group * cfg.blk),
                            dtype=np.float16)
        gg, ww = np.meshgrid(np.arange(cfg.n_groups),
                             np.arange(cfg.group * cfg.blk), indexing="ij")
        grow = c * cfg.rows_per_core + gg * cfg.group * cfg.blk + ww
        valid = grow < (c + 1) * cfg.rows_per_core
        valid &= grow < cfg.out_f
        bias_arr[gg[valid], 0, ww[valid]] = bias[grow[valid]].astype(
            np.float16)

        per_core.append({
            "xp": xp,
            "iota": iota,
            "idx": idx_full,
            "rv": rv,
            "biasb": bias_arr,
        })
    return per_core


def build_program(cfg, enable_asserts=False, debug=False):
    import concourse.bacc as bacc
    import concourse.mybir as mybir
    import concourse.tile as tile

    f16 = mybir.dt.float16
    f32 = mybir.dt.float32
    i16 = mybir.dt.int16

    nc = bacc.Bacc("TRN2", target_bir_lowering=False, debug=debug,
                   enable_asserts=enable_asserts, num_devices=cfg.n_cores,
                   num_swdge_queues=4)

    xp_d = nc.dram_tensor("xp", (cfg.in_f, cfg.xpad), f16, kind="ExternalInput")
    iota_d = nc.dram_tensor("iota", (128, 128), f16, kind="ExternalInput")
    idx_d = nc.dram_tensor("idx", (cfg.n_groups, 128,
                                   cfg.n_ranges * (cfg.group * cfg.cpr * 8)),
                           i16, kind="ExternalInput")
    rv_d = nc.dram_tensor("rv", (cfg.n_groups, 128, cfg.rv_w), f16,
                          kind="ExternalInput")
    bias_d = nc.dram_tensor("biasb", (cfg.n_groups, 1, cfg.group * cfg.blk),
                            f16, kind="ExternalInput")
    out_d = nc.dram_tensor("out_t", (cfg.out_rows, cfg.batch), f32,
                           kind="ExternalOutput")

    npart_w = cfg.group * cfg.cpr * 8          # idx free width per range

    with tile.TileContext(nc, num_cores=cfg.n_cores) as tc:
        with (
            tc.tile_pool(name="const", bufs=1) as cp,
            tc.tile_pool(name="meta", bufs=3) as mp_meta,
            tc.tile_pool(name="gath", bufs=4) as gp,
            tc.tile_pool(name="mtile", bufs=6) as mp,
            tc.tile_pool(name="ostage", bufs=2) as op,
            tc.tile_pool(name="ps", bufs=8, space="PSUM") as pp,
        ):
            iota_t = cp.tile([128, 128], f16)
            nc.sync.dma_start(out=iota_t[:], in_=iota_d[:, :])
            ones_t = cp.tile([1, cfg.batch], f16)
            nc.vector.memset(ones_t[:], 1.0)

            for g in range(cfg.n_groups):
                idx_t = mp_meta.tile([128, cfg.n_ranges * npart_w], i16,
                                     tag="idx")
                nc.sync.dma_start(out=idx_t[:], in_=idx_d[g])
                rv_t = mp_meta.tile([128, cfg.rv_w], f16, tag="rv")
                nc.sync.dma_start(out=rv_t[:], in_=rv_d[g])
                bias_t = mp_meta.tile([1, cfg.group * cfg.blk], f16,
                                      tag="bias")
                nc.sync.dma_start(out=bias_t[:], in_=bias_d[g])

                gath = gp.tile([128, cfg.slots_pg, cfg.xpad], f16, tag="g")
                for r in range(cfg.n_ranges):
                    lo = r * cfg.range_w
                    hi = min(lo + cfg.range_w, cfg.in_f)
                    nc.gpsimd.dma_gather(
                        out_ap=gath[:, r * cfg.group * cfg.cpr:
                                    (r + 1) * cfg.group * cfg.cpr, :],
                        in_ap=xp_d[lo:hi, :],
                        idxs_ap=idx_t[:, r * npart_w:(r + 1) * npart_w],
                        num_idxs=cfg.group * cfg.cpr * 128,
                        num_idxs_reg=cfg.group * cfg.cpr * 128,
                        elem_size=cfg.xpad,
                        # one packet per descriptor: a coalesced stream of
                        # >64 descriptors/engine aborts the SDMA engine
                        single_packet=False,
                        # each queue's descriptors are generated by a
                        # dedicated Q7 core pair -> 4x parallel desc-gen
                        queue_num=r % 4,
                    )

                import concourse.bass as bass_mod
                # gath[p, (r,j,k), :B] *= vals[p, (j,r,k)] in place — one DVE
                # op per range covering the whole group
                for r in range(cfg.n_ranges):
                    g0 = gath[:, r * cfg.group * cfg.cpr, :cfg.batch]
                    gsec = bass_mod.AP(
                        g0.tensor, g0.offset,
                        [g0.ap[0], [cfg.xpad, cfg.group * cfg.cpr],
                         [1, cfg.batch]])
                    v0 = rv_t[:, cfg.cpt + r * cfg.cpr:cfg.cpt + r * cfg.cpr + 1]
                    vals_bcast = bass_mod.AP(
                        v0.tensor, v0.offset,
                        [v0.ap[0], [2 * cfg.cpt, cfg.group], [1, cfg.cpr],
                         [0, cfg.batch]])
                    nc.vector.tensor_tensor(
                        out=gsec, in0=gsec, in1=vals_bcast,
                        op=mybir.AluOpType.mult)

                for j in range(cfg.group):
                    b = g * cfg.group + j
                    # M_eq[p, kk, m] = (rows[p, kk] == m), one DVE op per block
                    meq = mp.tile([128, cfg.cpt, 128], f16, tag="meq")
                    rows_ap = rv_t[:, j * 2 * cfg.cpt:j * 2 * cfg.cpt + cfg.cpt]
                    rows_bcast = rows_ap.to_broadcast([128, cfg.cpt, 128])
                    i0 = iota_t[:]
                    iota_rep = bass_mod.AP(
                        i0.tensor, i0.offset,
                        [i0.ap[0], [0, cfg.cpt], [1, 128]])
                    nc.vector.tensor_tensor(
                        out=meq[:], in0=rows_bcast, in1=iota_rep,
                        op=mybir.AluOpType.is_equal)

                    ps = pp.tile([128, cfg.batch], f32, tag="ps")
                    for kk in range(cfg.cpt):
                        r, k = divmod(kk, cfg.cpr)
                        slot = (r * cfg.group + j) * cfg.cpr + k
                        nc.tensor.matmul(
                            out=ps[:],
                            lhsT=meq[:, kk, :],
                            rhs=gath[:, slot, :cfg.batch],
                            start=(kk == 0),
                            stop=False,
                        )
                    # bias via rank-1 matmul: psum[m, :] += bias[m] * 1
                    nc.tensor.matmul(
                        out=ps[:],
                        lhsT=bias_t[0:1, j * cfg.blk:(j + 1) * cfg.blk],
                        rhs=ones_t[0:1, :],
                        start=False,
                        stop=True,
                    )
                    o_t = op.tile([128, cfg.batch], f32, tag="o")
                    nc.scalar.activation(
                        out=o_t[:], in_=ps[:],
                        func=mybir.ActivationFunctionType.Copy)
                    nc.sync.dma_start(
                        out=out_d[b * cfg.blk:(b + 1) * cfg.blk, :],
                        in_=o_t[:],
                    )

    nc.compile()
    return nc


def compute_cpr(cfg_like, rows, cols):
    """Global max chunks per (core, block, range)."""
    rows = np.asarray(rows).astype(np.int64)
    cols = np.asarray(cols).astype(np.int64)
    mx = 1
    for c in range(cfg_like["n_cores"]):
        rpc = cfg_like["rows_per_core"]
        e0, e1 = np.searchsorted(rows, [c * rpc, (c + 1) * rpc])
        r_loc = rows[e0:e1] - c * rpc
        key = (r_loc // cfg_like["blk"]) * cfg_like["n_ranges"] + \
            cols[e0:e1] // cfg_like["range_w"]
        nb = _cdiv(rpc, cfg_like["blk"])
        counts = np.bincount(key, minlength=nb * cfg_like["n_ranges"])
        mx = max(mx, int(_cdiv(int(counts.max()), 128)))
    return mx


LAST_RESULT = None  # BassKernelResults of the most recent kernel() call


def kernel(x, values, bias, rows, cols):
    global LAST_RESULT
    from concourse.bass_utils import run_bass_kernel_spmd

    rows_in = np.asarray(rows)
    cols_in = np.asarray(cols)

    cpr = compute_cpr(dict(n_cores=N_CORES, rows_per_core=ROWS_PER_CORE,
                           blk=BLK, n_ranges=N_RANGES, range_w=RANGE_W),
                      rows_in, cols_in)
    cfg = Cfg(IN_F, OUT_F, B, N_CORES, ROWS_PER_CORE, GROUP, N_RANGES,
              RANGE_W, cpr, xpad=XPAD, blk=BLK)

    per_core = prep_host_data(cfg, x, values, bias, rows_in, cols_in)
    nc = build_program(cfg)
    res = run_bass_kernel_spmd(nc, per_core, core_ids=list(range(N_CORES)))
    LAST_RESULT = res

    parts = [res.results[c]["out_t"][:ROWS_PER_CORE] for c in range(N_CORES)]
    out_t = np.concatenate(parts, axis=0)       # (OUT_F, B) f32
    return np.ascontiguousarray(out_t.T)        # (B, OUT_F) f32



# revision 9
# speedup vs baseline: 1.0059x; 1.0059x over previous
"""Trainium2 Bass kernel for nn_BaseSparseConn (gnn_message_passing).

Computes out = x @ conn + bias where conn is given in COO form
(rows = dst, cols = src of the transposed matrix):
    out.T[r, :] = sum_{e: rows[e]==r} values[e] * x[:, cols[e]]  + bias[r]

Strategy (8 NeuronCores, SPMD — one NEFF, per-core data):
  - Row-partition the output: core c owns output rows [c*12500, (c+1)*12500).
  - Per core, rows are processed in 98 blocks of 128 rows.  A block's edges
    (avg ~2048) are fetched with dma_gather (SWDGE) from a zero-padded fp16
    copy of x^T laid out as (IN_F, 128) so each gather element is 256 B.
    dma_gather requires int16 indices, so each block's edges are bucketed
    into 4 column ranges of 25000 and padded to a fixed chunk count.
  - Scatter-add into the 128 output rows of a block is a one-hot matmul:
    one batched DVE tensor_tensor builds M_eq[p, kk, m] = (rows[p,kk] == m)
    per block, values are multiplied into the gathered data in place (one
    DVE op per range covering the whole group), and the PE accumulates
    psum[128 rows, 64 batch] += M_eq[:,kk,:].T @ gathered across chunks.
    Gathers run on SWDGE queues 0-3 so four Q7 core pairs generate
    descriptors concurrently.
  - Bias is a final rank-1 matmul into PSUM; the Scalar engine copies
    PSUM->SBUF and the result is DMA'd out.
"""

import numpy as np

# Problem constants (hardcoded per the harness contract)
B = 64
IN_F = 100000
OUT_F = 100000
N_CORES = 8

# Sharding / layout constants
ROWS_PER_CORE = OUT_F // N_CORES  # 12500
BLK = 128
GROUP = 7                         # blocks per gather group (98 = 14*7)
N_RANGES = 4
RANGE_W = 25000                   # int16 gather index bound (< 32768)
XPAD = 128                        # padded batch so gather elem = 256 B fp16


def _cdiv(a, b):
    return -(-a // b)


class Cfg:
    """Geometry shared between host-side data prep and the device program."""

    def __init__(self, in_f, out_f, batch, n_cores, rows_per_core, group,
                 n_ranges, range_w, cpr, xpad=128, blk=128):
        assert range_w <= 32768
        assert rows_per_core % blk == 0 or True
        self.in_f = in_f
        self.out_f = out_f
        self.batch = batch
        self.n_cores = n_cores
        self.rows_per_core = rows_per_core
        self.blk = blk
        self.group = group
        self.n_ranges = n_ranges
        self.range_w = range_w
        assert n_ranges * range_w >= in_f
        self.cpr = cpr                        # chunks per (block, range)
        self.xpad = xpad
        self.n_blocks = _cdiv(rows_per_core, blk)       # blocks per core
        assert self.n_blocks % group == 0, (self.n_blocks, group)
        self.n_groups = self.n_blocks // group
        self.cpt = n_ranges * cpr             # chunks per block
        self.slots_pg = n_ranges * group * cpr  # gather slots per group
        self.idx_w = self.slots_pg * 8        # idx free-dim per group (int16)
        self.rv_w = group * self.cpt          # rows free-dim per group
        self.out_rows = self.n_blocks * blk   # padded output rows per core


def prep_host_data(cfg, x, values, bias, rows, cols):
    """Shard + lay out inputs for the device program.

    Returns (shared_inputs, per_core_inputs).
    """
    rows = np.asarray(rows).astype(np.int64)
    cols = np.asarray(cols).astype(np.int64)
    values = np.asarray(values, dtype=np.float32)
    bias = np.asarray(bias, dtype=np.float32)
    x = np.asarray(x, dtype=np.float32)

    # zero-padded fp16 x^T: row i = x[:, i] padded to xpad columns
    xp = np.zeros((cfg.in_f, cfg.xpad), dtype=np.float16)
    xp[:, :cfg.batch] = x.T.astype(np.float16)

    # contiguous iota matrix: row = [0..127] repeated cpt times (is_equal in0)
    iota = np.tile(np.arange(128, dtype=np.float16), (128, cfg.cpt))

    per_core = []
    for c in range(cfg.n_cores):
        e0, e1 = np.searchsorted(rows, [c * cfg.rows_per_core,
                                        (c + 1) * cfg.rows_per_core])
        r_loc = (rows[e0:e1] - c * cfg.rows_per_core).astype(np.int64)
        col = cols[e0:e1]
        val = values[e0:e1]

        blk_id = r_loc // cfg.blk
        rng_id = col // cfg.range_w
        key = blk_id * cfg.n_ranges + rng_id
        order = np.argsort(key, kind="stable")
        key_s = key[order]
        col_s = col[order]
        val_s = val[order]
        row_s = (r_loc - blk_id * cfg.blk)[order]       # 0..127 within block

        counts = np.bincount(key_s, minlength=cfg.n_blocks * cfg.n_ranges)
        limit = cfg.cpr * 128
        assert counts.max() <= limit, (counts.max(), limit)
        starts = np.concatenate([[0], np.cumsum(counts)[:-1]])
        # position of each edge within its (block, range) bucket
        q = np.arange(len(key_s)) - starts[key_s]

        b_s = key_s // cfg.n_ranges
        r_s = key_s % cfg.n_ranges
        g_s = b_s // cfg.group
        j_s = b_s % cfg.group

        # ---- gather index array, 8x replicated across the 128 partitions.
        # One gather per (group, range): batch of group*cpr*128 indices,
        # element i lives at [i % 16, base + i // 16].
        npart_w = cfg.group * cfg.cpr * 128 // 16        # per-range free width
        # Padding slots repeat the bucket's last real index (HBM row-hit
        # instead of a cold read of row 0); value is 0 so they contribute 0.
        pad_idx = np.zeros((cfg.n_groups, cfg.n_ranges, cfg.group, cfg.cpr * 128),
                           dtype=np.int16)
        lastidx = np.zeros(cfg.n_groups * cfg.n_ranges * cfg.group,
                           dtype=np.int16)
        flatkey = (g_s * cfg.n_ranges + r_s) * cfg.group + j_s
        lastidx[flatkey] = (col_s - r_s * cfg.range_w).astype(np.int16)
        pad_idx[:] = lastidx.reshape(cfg.n_groups, cfg.n_ranges,
                                     cfg.group)[..., None]
        pad_idx = pad_idx.reshape(cfg.n_groups, cfg.n_ranges,
                                  cfg.group * cfg.cpr * 128)
        # scatter into the 16-partition wrap layout
        idx16 = np.empty((cfg.n_groups, 16, cfg.n_ranges * npart_w),
                         dtype=np.int16)
        for rr in range(cfg.n_ranges):
            blk16 = pad_idx[:, rr].reshape(cfg.n_groups, npart_w, 16)
            idx16[:, :, rr * npart_w:(rr + 1) * npart_w] = \
                blk16.transpose(0, 2, 1)
        i_in_gather = j_s * (cfg.cpr * 128) + q
        idx16[g_s, i_in_gather % 16,
              r_s * npart_w + i_in_gather // 16] = (col_s - r_s * cfg.range_w
                                                    ).astype(np.int16)
        idx_full = np.tile(idx16, (1, 8, 1))             # replicate to 128 parts

        # ---- rows array: (n_groups, 128, group*cpt) fp16, j-major chunk order
        # (meq reads a contiguous cpt window per block)
        rv = np.zeros((cfg.n_groups, 128, cfg.rv_w), dtype=np.float16)
        kk = r_s * cfg.cpr + q // 128                    # chunk id in block
        p = q % 128
        rv[g_s, p, j_s * cfg.cpt + kk] = row_s.astype(np.float16)
        # ---- values array: (n_groups, 128, slots_pg) fp16 in gather-slot
        # order (r-major) so one contiguous DVE mult covers the whole group.
        # Pad slots keep value 0 -> stale gathered data contributes nothing.
        vb = np.zeros((cfg.n_groups, 128, cfg.slots_pg), dtype=np.float16)
        slot_g = (r_s * cfg.group + j_s) * cfg.cpr + q // 128
        vb[g_s, p, slot_g] = val_s.astype(np.float16)

        # ---- bias array: (n_groups, 1, group*blk) fp16 (rank-1 matmul row)
        bias_arr = np.zeros((cfg.n_groups, 1, cfg.group * cfg.blk),
                            dtype=np.float16)
        gg, ww = np.meshgrid(np.arange(cfg.n_groups),
                             np.arange(cfg.group * cfg.blk), indexing="ij")
        grow = c * cfg.rows_per_core + gg * cfg.group * cfg.blk + ww
        valid = grow < (c + 1) * cfg.rows_per_core
        valid &= grow < cfg.out_f
        bias_arr[gg[valid], 0, ww[valid]] = bias[grow[valid]].astype(
            np.float16)

        per_core.append({
            "xp": xp,
            "iota": iota,
            "idx": idx_full,
            "rv": rv,
            "valsb": vb,
            "biasb": bias_arr,
        })
    return per_core


def build_program(cfg, enable_asserts=False, debug=False):
    import concourse.bacc as bacc
    import concourse.mybir as mybir
    import concourse.tile as tile

    f16 = mybir.dt.float16
    f32 = mybir.dt.float32
    i16 = mybir.dt.int16

    nc = bacc.Bacc("TRN2", target_bir_lowering=False, debug=debug,
                   enable_asserts=enable_asserts, num_devices=cfg.n_cores,
                   num_swdge_queues=4)

    xp_d = nc.dram_tensor("xp", (cfg.in_f, cfg.xpad), f16, kind="ExternalInput")
    iota_d = nc.dram_tensor("iota", (128, cfg.cpt * 128), f16,
                            kind="ExternalInput")
    idx_d = nc.dram_tensor("idx", (cfg.n_groups, 128,
                                   cfg.n_ranges * (cfg.group * cfg.cpr * 8)),
                           i16, kind="ExternalInput")
    rv_d = nc.dram_tensor("rv", (cfg.n_groups, 128, cfg.rv_w), f16,
                          kind="ExternalInput")
    vals_d = nc.dram_tensor("valsb", (cfg.n_groups, 128, cfg.slots_pg), f16,
                            kind="ExternalInput")
    bias_d = nc.dram_tensor("biasb", (cfg.n_groups, 1, cfg.group * cfg.blk),
                            f16, kind="ExternalInput")
    out_d = nc.dram_tensor("out_t", (cfg.out_rows, cfg.batch), f32,
                           kind="ExternalOutput")

    npart_w = cfg.group * cfg.cpr * 8          # idx free width per range

    with tile.TileContext(nc, num_cores=cfg.n_cores) as tc:
        with (
            tc.tile_pool(name="const", bufs=1) as cp,
            tc.tile_pool(name="meta", bufs=3) as mp_meta,
            tc.tile_pool(name="gath", bufs=4) as gp,
            tc.tile_pool(name="mtile", bufs=6) as mp,
            tc.tile_pool(name="ostage", bufs=2) as op,
            tc.tile_pool(name="ps", bufs=8, space="PSUM") as pp,
        ):
            iota_t = cp.tile([128, cfg.cpt * 128], f16)
            nc.sync.dma_start(out=iota_t[:], in_=iota_d[:, :])
            ones_t = cp.tile([1, cfg.batch], f16)
            nc.vector.memset(ones_t[:], 1.0)

            for g in range(cfg.n_groups):
                idx_t = mp_meta.tile([128, cfg.n_ranges * npart_w], i16,
                                     tag="idx")
                nc.sync.dma_start(out=idx_t[:], in_=idx_d[g])
                rv_t = mp_meta.tile([128, cfg.rv_w], f16, tag="rv")
                nc.sync.dma_start(out=rv_t[:], in_=rv_d[g])
                vals_t = mp_meta.tile([128, cfg.slots_pg], f16, tag="vals")
                nc.sync.dma_start(out=vals_t[:], in_=vals_d[g])
                bias_t = mp_meta.tile([1, cfg.group * cfg.blk], f16,
                                      tag="bias")
                nc.sync.dma_start(out=bias_t[:], in_=bias_d[g])

                gath = gp.tile([128, cfg.slots_pg, cfg.xpad], f16, tag="g")
                for r in range(cfg.n_ranges):
                    lo = r * cfg.range_w
                    hi = min(lo + cfg.range_w, cfg.in_f)
                    nc.gpsimd.dma_gather(
                        out_ap=gath[:, r * cfg.group * cfg.cpr:
                                    (r + 1) * cfg.group * cfg.cpr, :],
                        in_ap=xp_d[lo:hi, :],
                        idxs_ap=idx_t[:, r * npart_w:(r + 1) * npart_w],
                        num_idxs=cfg.group * cfg.cpr * 128,
                        num_idxs_reg=cfg.group * cfg.cpr * 128,
                        elem_size=cfg.xpad,
                        # one packet per descriptor: a coalesced stream of
                        # >64 descriptors/engine aborts the SDMA engine
                        single_packet=False,
                        # each queue's descriptors are generated by a
                        # dedicated Q7 core pair -> 4x parallel desc-gen
                        queue_num=r % 4,
                    )

                import concourse.bass as bass_mod
                # gath[p, s, :] *= vals[p, s] in place — ONE contiguous
                # full-width DVE op per group (pad lanes are zero in xp)
                g0 = gath[:, 0, :]
                gfull = bass_mod.AP(
                    g0.tensor, g0.offset,
                    [g0.ap[0], [cfg.xpad, cfg.slots_pg], [1, cfg.xpad]])
                v0 = vals_t[:, 0:1]
                vals_bcast = bass_mod.AP(
                    v0.tensor, v0.offset,
                    [v0.ap[0], [1, cfg.slots_pg], [0, cfg.xpad]])
                nc.vector.tensor_tensor(
                    out=gfull, in0=gfull, in1=vals_bcast,
                    op=mybir.AluOpType.mult)

                for j in range(cfg.group):
                    b = g * cfg.group + j
                    # M_eq[p, kk, m] = (iota[p, kk*128+m] == rows[p, kk]),
                    # one DVE op per block; in0 unit-stride contiguous
                    meq = mp.tile([128, cfg.cpt, 128], f16, tag="meq")
                    r0 = rv_t[:, j * cfg.cpt:j * cfg.cpt + 1]
                    rows_bcast = bass_mod.AP(
                        r0.tensor, r0.offset,
                        [r0.ap[0], [1, cfg.cpt], [0, 128]])
                    i0 = iota_t[:]
                    iota_rep = bass_mod.AP(
                        i0.tensor, i0.offset,
                        [i0.ap[0], [128, cfg.cpt], [1, 128]])
                    nc.vector.tensor_tensor(
                        out=meq[:], in0=iota_rep, in1=rows_bcast,
                        op=mybir.AluOpType.is_equal)

                    ps = pp.tile([128, cfg.batch], f32, tag="ps")
                    for kk in range(cfg.cpt):
                        r, k = divmod(kk, cfg.cpr)
                        slot = (r * cfg.group + j) * cfg.cpr + k
                        nc.tensor.matmul(
                            out=ps[:],
                            lhsT=meq[:, kk, :],
                            rhs=gath[:, slot, :cfg.batch],
                            start=(kk == 0),
                            stop=False,
                        )
                    # bias via rank-1 matmul: psum[m, :] += bias[m] * 1
                    nc.tensor.matmul(
                        out=ps[:],
                        lhsT=bias_t[0:1, j * cfg.blk:(j + 1) * cfg.blk],
                        rhs=ones_t[0:1, :],
                        start=False,
                        stop=True,
                    )
                    o_t = op.tile([128, cfg.batch], f32, tag="o")
                    nc.scalar.activation(
                        out=o_t[:], in_=ps[:],
                        func=mybir.ActivationFunctionType.Copy)
                    nc.sync.dma_start(
                        out=out_d[b * cfg.blk:(b + 1) * cfg.blk, :],
                        in_=o_t[:],
                    )

    nc.compile()
    return nc


def compute_cpr(cfg_like, rows, cols):
    """Global max chunks per (core, block, range)."""
    rows = np.asarray(rows).astype(np.int64)
    cols = np.asarray(cols).astype(np.int64)
    mx = 1
    for c in range(cfg_like["n_cores"]):
        rpc = cfg_like["rows_per_core"]
        e0, e1 = np.searchsorted(rows, [c * rpc, (c + 1) * rpc])
        r_loc = rows[e0:e1] - c * rpc
        key = (r_loc // cfg_like["blk"]) * cfg_like["n_ranges"] + \
            cols[e0:e1] // cfg_like["range_w"]
        nb = _cdiv(rpc, cfg_like["blk"])
        counts = np.bincount(key, minlength=nb * cfg_like["n_ranges"])
        mx = max(mx, int(_cdiv(int(counts.max()), 128)))
    return mx


LAST_RESULT = None  # BassKernelResults of the most recent kernel() call


def kernel(x, values, bias, rows, cols):
    global LAST_RESULT
    from concourse.bass_utils import run_bass_kernel_spmd

    rows_in = np.asarray(rows)
    cols_in = np.asarray(cols)

    cpr = compute_cpr(dict(n_cores=N_CORES, rows_per_core=ROWS_PER_CORE,
                           blk=BLK, n_ranges=N_RANGES, range_w=RANGE_W),
                      rows_in, cols_in)
    cfg = Cfg(IN_F, OUT_F, B, N_CORES, ROWS_PER_CORE, GROUP, N_RANGES,
              RANGE_W, cpr, xpad=XPAD, blk=BLK)

    per_core = prep_host_data(cfg, x, values, bias, rows_in, cols_in)
    nc = build_program(cfg)
    res = run_bass_kernel_spmd(nc, per_core, core_ids=list(range(N_CORES)))
    LAST_RESULT = res

    parts = [res.results[c]["out_t"][:ROWS_PER_CORE] for c in range(N_CORES)]
    out_t = np.concatenate(parts, axis=0)       # (OUT_F, B) f32
    return np.ascontiguousarray(out_t.T)        # (B, OUT_F) f32

